# revision 2
# baseline (speedup 1.0000x reference)
"""Trainium2 Bass kernel for the Consis_Reg MSE loss.

Reference semantics (N=8192, D=512, C=64 classes):
    S[i,j]    = ||a_i - a_j||^2
    per_row_i = sum_{j: t_j == t_i} S[i,j] / cnt_{t_i}
    loss      = sum_i per_row_i

Class-aggregation identity (exact in real arithmetic):
    sum_{j in c} S[i,j] = cnt_c * ||a_i||^2 + sumSq_c - 2 a_i . sumA_c
    =>  loss = 2 * ( sum_c sumSq_c - sum_c ||sumA_c||^2 / cnt_c )
where, per class c:
    sumA_c  = sum_{i in c} a_i          (vector in R^D)
    sumSq_c = sum_{i in c} ||a_i||^2
    cnt_c   = |{i : t_i == c}|

Each of the 8 cores processes a 1024-row shard of A and produces
    osum [64, 512] = M^T @ A_shard           (M = one-hot of targets)
    ocs  [64, 2]   = M^T @ [ones | rowsumsq] (per-class count, sum of squares)
via PSUM-accumulated matmuls; the one-hot M is built on-device with
iota + is_equal. The host sums the 8 partials and folds them into the
final scalar (a ~33K-element reduction).
"""

import numpy as np

N, D, C = 8192, 512, 64
NCORES = 8
ROWS = N // NCORES  # rows per core
P = 128             # SBUF partitions
NT = ROWS // P      # row-tiles per core

_PROGRAM_CACHE = {}


def _build_program():
    import concourse.bacc as bacc
    import concourse.tile as tile
    from concourse import mybir

    f32 = mybir.dt.float32
    f32r = mybir.dt.float32r
    i32 = mybir.dt.int32

    nc = bacc.Bacc(
        "TRN2", target_bir_lowering=False, debug=False, num_devices=NCORES
    )
    a_dram = nc.dram_tensor("a", [NT, P, D], f32, kind="ExternalInput").ap()
    t_dram = nc.dram_tensor("t", [NT, P, 1], i32, kind="ExternalInput").ap()
    osum = nc.dram_tensor("osum", [C, D], f32, kind="ExternalOutput").ap()
    ocs = nc.dram_tensor("ocs", [C, 2], f32, kind="ExternalOutput").ap()

    with tile.TileContext(nc) as tc:
        with (
            tc.tile_pool(name="const", bufs=1) as cpool,
            tc.tile_pool(name="apool", bufs=4) as apool,
            tc.tile_pool(name="small", bufs=4) as spool,
            tc.tile_pool(name="sqscr", bufs=2) as qpool,
            tc.tile_pool(name="outp", bufs=1) as opool,
            tc.tile_pool(name="psum", bufs=1, space="PSUM") as pspool,
        ):
            iota_i = cpool.tile([P, C], i32)
            nc.gpsimd.iota(iota_i, pattern=[[1, C]], base=0, channel_multiplier=0)
            iota_f = cpool.tile([P, C], f32)
            nc.vector.tensor_copy(iota_f, iota_i)

            psum_s = pspool.tile([C, D], f32)
            psum_c = pspool.tile([C, 2], f32)

            for i in range(NT):
                a_tile = apool.tile([P, D], f32)
                nc.sync.dma_start(out=a_tile, in_=a_dram[i])

                t_i = spool.tile([P, 1], i32)
                nc.sync.dma_start(out=t_i, in_=t_dram[i])
                t_f = spool.tile([P, 1], f32)
                nc.vector.tensor_copy(t_f, t_i)

                # one-hot row block M[p, c] = (t[p] == c)
                m_tile = spool.tile([P, C], f32)
                nc.vector.tensor_scalar(
                    m_tile, iota_f, t_f, None, mybir.AluOpType.is_equal
                )

                # rhs2 = [ones | rowwise sum of a^2]
                rhs2 = spool.tile([P, 2], f32)
                nc.vector.memset(rhs2[:, 0:1], 1.0)
                sq_scr = qpool.tile([P, D], f32)
                nc.scalar.activation(
                    sq_scr,
                    a_tile,
                    mybir.ActivationFunctionType.Square,
                    accum_out=rhs2[:, 1:2],
                )

                nc.tensor.matmul(
                    psum_s,
                    lhsT=m_tile,
                    rhs=a_tile,
                    start=(i == 0),
                    stop=(i == NT - 1),
                )
                nc.tensor.matmul(
                    psum_c,
                    lhsT=m_tile,
                    rhs=rhs2,
                    start=(i == 0),
                    stop=(i == NT - 1),
                )

            osum_sb = opool.tile([C, D], f32)
            nc.vector.tensor_copy(osum_sb, psum_s)
            nc.sync.dma_start(out=osum, in_=osum_sb)
            ocs_sb = opool.tile([C, 2], f32)
            nc.vector.tensor_copy(ocs_sb, psum_c)
            nc.sync.dma_start(out=ocs, in_=ocs_sb)

    nc.compile()
    return nc


def get_program():
    if "nc" not in _PROGRAM_CACHE:
        _PROGRAM_CACHE["nc"] = _build_program()
    return _PROGRAM_CACHE["nc"]


def make_in_maps(representations, targets):
    A = np.ascontiguousarray(np.asarray(representations, dtype=np.float32))
    t = np.ascontiguousarray(np.asarray(targets).astype(np.int32))
    in_maps = []
    for core in range(NCORES):
        a_sh = A[core * ROWS : (core + 1) * ROWS].reshape(NT, P, D)
        t_sh = t[core * ROWS : (core + 1) * ROWS].reshape(NT, P, 1)
        in_maps.append({"a": a_sh, "t": t_sh})
    return in_maps


def combine_partials(results):
    sums = np.zeros((C, D), np.float64)
    cs = np.zeros((C, 2), np.float64)
    for r in results:
        sums += r["osum"].astype(np.float64)
        cs += r["ocs"].astype(np.float64)
    cnt = cs[:, 0]
    total_sumsq = cs[:, 1].sum()
    loss = 2.0 * (total_sumsq - ((sums * sums).sum(axis=1) / cnt).sum())
    return np.float32(loss)


def kernel(representations, targets):
    from concourse.bass_utils import run_bass_kernel_spmd

    nc = get_program()
    in_maps = make_in_maps(representations, targets)
    res = run_bass_kernel_spmd(nc, in_maps, list(range(NCORES)))
    return combine_partials(res.results)


# revision 11
# speedup vs baseline: 1.3446x; 1.3446x over previous
"""Trainium2 Bass kernel for the Consis_Reg MSE loss.

Reference semantics (N=8192, D=512, C=64 classes):
    S[i,j]    = ||a_i - a_j||^2
    per_row_i = sum_{j: t_j == t_i} S[i,j] / cnt_{t_i}
    loss      = sum_i per_row_i

Class-aggregation identity (exact in real arithmetic):
    sum_{j in c} S[i,j] = cnt_c * ||a_i||^2 + sumSq_c - 2 a_i . sumA_c
    =>  loss = 2 * ( total_sumsq - sum_c ||sumA_c||^2 / cnt_c )
where, per class c:
    sumA_c  = sum_{i in c} a_i          (vector in R^D)
    cnt_c   = |{i : t_i == c}|
and total_sumsq = sum_i ||a_i||^2.

Each of the 8 cores processes a 1024-row shard of A:
    osum [64, 512] = M^T @ A_shard   (M = one-hot of targets, PSUM-accumulated
                                      bf16 matmuls; exact since M is 0/1 and
                                      the bf16 rounding of A only perturbs the
                                      small ||sumA_c||^2 correction term)
    ocnt [1, 64]   = per-class count (DVE reduce + GpSimd partition reduce)
    osq  [1, 1]    = sum of squares of the shard (DVE mult+reduce, fp32)
The host sums the 8 partials and folds them into the final scalar.

Rows are assigned to SBUF partitions in contiguous blocks (partition p gets
rows p*8..p*8+7 of the shard) so input DMAs move 8-16KB contiguous chunks per
partition instead of 2KB packets; the matmul contraction is invariant to row
order because the one-hot rows are permuted identically.
"""

import numpy as np

N, D, C = 8192, 512, 64
NCORES = 8
ROWS = N // NCORES  # rows per core
P = 128             # SBUF partitions
NT = ROWS // P      # row-tiles per core (rows per partition)
HALF = NT // 2

_PROGRAM_CACHE = {}

# build-time toggles for hardware bisection
USE_PARTITION_REDUCE = True
USE_ACT_CAST = True


def _build_program():
    import concourse.bacc as bacc
    import concourse.tile as tile
    from concourse import bass_isa, mybir

    f32 = mybir.dt.float32
    bf16 = mybir.dt.bfloat16
    i32 = mybir.dt.int32

    nc = bacc.Bacc(
        "TRN2", target_bir_lowering=False, debug=False, num_devices=NCORES
    )
    a_dram = nc.dram_tensor("a", [P, NT, D], f32, kind="ExternalInput").ap()
    t_dram = nc.dram_tensor("t", [P, NT], i32, kind="ExternalInput").ap()
    osum = nc.dram_tensor("osum", [C, D], f32, kind="ExternalOutput").ap()
    if USE_PARTITION_REDUCE:
        ocnt = nc.dram_tensor("ocnt", [1, C], f32, kind="ExternalOutput").ap()
        osq = nc.dram_tensor("osq", [1, 1], f32, kind="ExternalOutput").ap()
    else:
        ocnt = nc.dram_tensor("ocnt", [P, C], f32, kind="ExternalOutput").ap()
        osq = nc.dram_tensor("osq", [P, 1], f32, kind="ExternalOutput").ap()

    with tile.TileContext(nc) as tc:
        with (
            tc.tile_pool(name="big", bufs=1) as big,
            tc.tile_pool(name="small", bufs=1) as small,
            tc.tile_pool(name="psum", bufs=1, space="PSUM") as pspool,
        ):
            iota_i = small.tile([P, C], i32)
            nc.gpsimd.iota(iota_i, pattern=[[1, C]], base=0, channel_multiplier=0)
            iota_f = small.tile([P, C], f32)
            nc.vector.tensor_copy(iota_f, iota_i)

            t_sb = small.tile([P, NT], i32)
            nc.sync.dma_start(out=t_sb, in_=t_dram)
            t_f = small.tile([P, NT], f32)
            nc.vector.tensor_copy(t_f, t_sb)

            a_sb = big.tile([P, NT, D], f32)
            for h in range(2):
                lo, hi = h * HALF, (h + 1) * HALF
                nc.sync.dma_start(out=a_sb[:, lo:hi, :], in_=a_dram[:, lo:hi, :])

            # one-hot blocks M[p, r, c] = (t[p, r] == c), bf16 for the matmul
            m_all = big.tile([P, NT, C], bf16)
            for r in range(NT):
                nc.vector.tensor_scalar(
                    m_all[:, r, :],
                    iota_f,
                    t_f[:, r : r + 1],
                    None,
                    mybir.AluOpType.is_equal,
                )

            # bf16 copy of A for the matmul (ACT engine, halves)
            a_bf = big.tile([P, NT, D], bf16)
            for h in range(2):
                lo, hi = h * HALF, (h + 1) * HALF
                if USE_ACT_CAST:
                    nc.scalar.copy(a_bf[:, lo:hi, :], a_sb[:, lo:hi, :])
                else:
                    nc.vector.tensor_copy(a_bf[:, lo:hi, :], a_sb[:, lo:hi, :])

            # per-partition sum of squares (fp32 square + row-sum in one
            # DVE op; tensor_tensor_reduce is broken on this runtime, so
            # use scalar_tensor_tensor: out=(a*1.0)*a, accum=rowsum(out))
            sq_scr = big.tile([P, HALF * D], f32)
            sqp = small.tile([P, 2], f32)
            for h in range(2):
                lo, hi = h * HALF, (h + 1) * HALF
                nc.vector.scalar_tensor_tensor(
                    out=sq_scr,
                    in0=a_sb[:, lo:hi, :].rearrange("p a d -> p (a d)"),
                    scalar=1.0,
                    in1=a_sb[:, lo:hi, :].rearrange("p a d -> p (a d)"),
                    op0=mybir.AluOpType.mult,
                    op1=mybir.AluOpType.mult,
                    accum_out=sqp[:, h : h + 1],
                )

            # PSUM-accumulated class sums: osum = sum_r M_r^T @ A_r
            psum_s = pspool.tile([C, D], f32)
            for r in range(NT):
                nc.tensor.matmul(
                    psum_s,
                    lhsT=m_all[:, r, :],
                    rhs=a_bf[:, r, :],
                    start=(r == 0),
                    stop=(r == NT - 1),
                )

            # counts: sum M over the NT axis (DVE), then over partitions
            cnt_sum = small.tile([P, C], f32)
            nc.vector.reduce_sum(
                cnt_sum,
                m_all.rearrange("p a c -> p c a"),
                axis=mybir.AxisListType.X,
            )
            if USE_PARTITION_REDUCE:
                cnt_red = small.tile([P, C], f32)
                nc.gpsimd.partition_all_reduce(
                    cnt_red, cnt_sum, channels=P, reduce_op=bass_isa.ReduceOp.add
                )
                nc.sync.dma_start(out=ocnt, in_=cnt_red[0:1, :])
            else:
                nc.sync.dma_start(out=ocnt, in_=cnt_sum)

            # total sumsq: reduce the two half-partials, then over partitions
            sq1 = small.tile([P, 1], f32)
            nc.vector.reduce_sum(sq1, sqp, axis=mybir.AxisListType.X)
            if USE_PARTITION_REDUCE:
                sq_red = small.tile([P, 1], f32)
                nc.gpsimd.partition_all_reduce(
                    sq_red, sq1, channels=P, reduce_op=bass_isa.ReduceOp.add
                )
                nc.sync.dma_start(out=osq, in_=sq_red[0:1, :])
            else:
                nc.sync.dma_start(out=osq, in_=sq1)

            osum_sb = small.tile([C, D], f32)
            nc.vector.tensor_copy(osum_sb, psum_s)
            nc.sync.dma_start(out=osum, in_=osum_sb)

    nc.compile()
    return nc


def get_program():
    if "nc" not in _PROGRAM_CACHE:
        _PROGRAM_CACHE["nc"] = _build_program()
    return _PROGRAM_CACHE["nc"]


def make_in_maps(representations, targets):
    A = np.ascontiguousarray(np.asarray(representations, dtype=np.float32))
    t = np.ascontiguousarray(np.asarray(targets).astype(np.int32))
    in_maps = []
    for core in range(NCORES):
        a_sh = A[core * ROWS : (core + 1) * ROWS].reshape(P, NT, D)
        t_sh = t[core * ROWS : (core + 1) * ROWS].reshape(P, NT)
        in_maps.append({"a": a_sh, "t": t_sh})
    return in_maps


def combine_partials(results):
    sums = np.zeros((C, D), np.float64)
    cnt = np.zeros(C, np.float64)
    total_sumsq = 0.0
    for r in results:
        sums += r["osum"].astype(np.float64)
        rc = r["ocnt"].astype(np.float64)
        cnt += rc[0] if rc.shape[0] == 1 else rc.sum(axis=0)
        rq = r["osq"].astype(np.float64)
        total_sumsq += float(rq[0, 0]) if rq.shape[0] == 1 else float(rq.sum())
    loss = 2.0 * (total_sumsq - ((sums * sums).sum(axis=1) / cnt).sum())
    return np.float32(loss)


def kernel(representations, targets):
    from concourse.bass_utils import run_bass_kernel_spmd

    nc = get_program()
    in_maps = make_in_maps(representations, targets)
    res = run_bass_kernel_spmd(nc, in_maps, list(range(NCORES)))
    return combine_partials(res.results)
